# revision 1
# baseline (speedup 1.0000x reference)
"""Trainium2 Bass kernel for CpuLstmModel (LSTM over basins).

Reference computation (per timestep t):
    x0    = relu(x_t @ W_in.T + b_in)                    # [G, H]
    gates = x0 @ w_ih.T + b_ih + h @ w_hh.T + b_hh       # [G, 4H]
    i,f,g,o = split(gates, 4)
    c = sigmoid(f)*c + sigmoid(i)*tanh(g)
    h = sigmoid(o)*tanh(c)
    y_t = h @ W_out.T + b_out                            # [G, 1]

Sharding: data-parallel over ngrid (4096 basins) across 8 cores; weights and
h/c state replicated. On-chip layout is feature-major ("transposed"):
activations live as [hid, basins] tiles so every matmul is
  out[gate_chunk(128), basins(512)] += W_chunk[k(128), gate_chunk(128)].T @ act[k(128), basins(512)]
with basins as the N=512 moving dim. Matmuls run in float32r.
"""

import numpy as np

import concourse.bass as bass
import concourse.mybir as mybir
import concourse.tile as tile
from concourse import bacc
from concourse.bass import ds, ts
from concourse.bass_utils import run_bass_kernel_spmd

NT = 365
NGRID = 4096
NX = 32
HID = 512
NY = 1
N_CORES = 8
G = NGRID // N_CORES  # basins per core = 512
KC = HID // 128  # hid chunks = 4
NGATE = 4 * HID  # 2048
MC = NGATE // 128  # gate chunks = 16

F32 = mybir.dt.float32
F32R = mybir.dt.float32r
AF = mybir.ActivationFunctionType

U = 28  # steps per hardware-loop iteration; must be EVEN (h ping-pong) and divide NT-1=364


def build_program(nt=NT, unroll=U, use_loop=True, staggered=False, psum6=False):
    nc = bacc.Bacc("TRN2", num_devices=N_CORES)

    xt_d = nc.dram_tensor("xt", [nt * NX, G], F32, kind="ExternalInput").ap()
    wih_d = nc.dram_tensor("wihT", [HID, NGATE], F32, kind="ExternalInput").ap()
    whh_d = nc.dram_tensor("whhT", [HID, NGATE], F32, kind="ExternalInput").ap()
    win_d = nc.dram_tensor("winT", [NX, HID], F32, kind="ExternalInput").ap()
    wout_d = nc.dram_tensor("woutC", [128, KC], F32, kind="ExternalInput").ap()
    bin_d = nc.dram_tensor("binC", [128, KC], F32, kind="ExternalInput").ap()
    bg_d = nc.dram_tensor("bgC", [128, MC], F32, kind="ExternalInput").ap()
    bout_d = nc.dram_tensor("bout", [1, 1], F32, kind="ExternalInput").ap()
    y_d = nc.dram_tensor("y", [nt, G], F32, kind="ExternalOutput").ap()

    with tile.TileContext(nc) as tc:
        with (
            tc.tile_pool(name="const", bufs=1) as cpool,
            tc.tile_pool(name="stag", bufs=2) as stag_pool,
            tc.tile_pool(name="xt", bufs=3) as xt_pool,
            tc.tile_pool(name="xtr", bufs=3) as xtr_pool,
            tc.tile_pool(name="x0", bufs=8) as x0_pool,
            tc.tile_pool(name="acts", bufs=10) as act_pool,
            tc.tile_pool(name="tmp", bufs=6) as tmp_pool,
            tc.tile_pool(name="state", bufs=1) as state_pool,
            tc.tile_pool(name="ysb", bufs=2) as y_pool,
            tc.tile_pool(name="gpsum", bufs=6 if psum6 else 5, space="PSUM") as gpsum,
            tc.tile_pool(name="xpsum", bufs=1 if psum6 else 2, space="PSUM") as xpsum,
            tc.tile_pool(name="ypsum", bufs=1, space="PSUM") as ypsum,
        ):
            # ---- load + round weights ----
            w_ih_r = cpool.tile([128, KC * NGATE], F32R, name="w_ih_r", tag="w_ih_r")
            w_hh_r = cpool.tile([128, KC * NGATE], F32R, name="w_hh_r", tag="w_hh_r")
            for k in range(KC):
                st = stag_pool.tile([128, NGATE], F32, name="st", tag="st")
                nc.gpsimd.dma_start(st[:], wih_d[ts(k, 128), :])
                nc.vector.tensor_copy(w_ih_r[:, ts(k, NGATE)], st[:])
                st2 = stag_pool.tile([128, NGATE], F32, name="st", tag="st")
                nc.gpsimd.dma_start(st2[:], whh_d[ts(k, 128), :])
                nc.vector.tensor_copy(w_hh_r[:, ts(k, NGATE)], st2[:])
            stw = stag_pool.tile([NX, HID], F32, name="stw", tag="stw")
            nc.gpsimd.dma_start(stw[:], win_d[:, :])
            w_in_r = cpool.tile([NX, HID], F32R, name="w_in_r", tag="w_in_r")
            nc.vector.tensor_copy(w_in_r[:], stw[:])
            sto = stag_pool.tile([128, KC], F32, name="sto", tag="sto")
            nc.gpsimd.dma_start(sto[:], wout_d[:, :])
            w_out_r = cpool.tile([128, KC], F32R, name="w_out_r", tag="w_out_r")
            nc.vector.tensor_copy(w_out_r[:], sto[:])
            b_in_sb = cpool.tile([128, KC], F32, name="b_in_sb", tag="b_in_sb")
            nc.gpsimd.dma_start(b_in_sb[:], bin_d[:, :])
            b_g_sb = cpool.tile([128, MC], F32, name="b_g_sb", tag="b_g_sb")
            nc.gpsimd.dma_start(b_g_sb[:], bg_d[:, :])
            b_out_sb = cpool.tile([1, 1], F32, name="b_out_sb", tag="b_out_sb")
            nc.gpsimd.dma_start(b_out_sb[:], bout_d[:, :])

            # ---- persistent state (h double-buffered: gate matmuls of step t
            # read h from step t-1 while the elementwise tail writes step t's) ----
            h_a = [state_pool.tile([128, G], F32R, name=f"ha{j}", tag=f"ha{j}") for j in range(KC)]
            h_b = [state_pool.tile([128, G], F32R, name=f"hb{j}", tag=f"hb{j}") for j in range(KC)]
            c_t = [state_pool.tile([128, G], F32, name=f"c{j}", tag=f"c{j}") for j in range(KC)]

            gate_funcs = [AF.Sigmoid, AF.Sigmoid, AF.Tanh, AF.Sigmoid]  # i, f, g, o

            def step(t, first, h_prev, h_cur):
                xt_sb = xt_pool.tile([NX, G], F32, name="xt_sb", tag="xt_sb")
                nc.gpsimd.dma_start(xt_sb[:], xt_d[ts(t, NX), :])
                xt_r = xtr_pool.tile([NX, G], F32R, name="xt_r", tag="xt_r")
                nc.vector.tensor_copy(xt_r[:], xt_sb[:])

                x0_r = []
                for m in range(KC):
                    xps = xpsum.tile([128, G], F32, name="xps", tag="xps")
                    nc.tensor.matmul(
                        xps[:], w_in_r[:, ts(m, 128)], xt_r[:], start=True, stop=True
                    )
                    xr = x0_pool.tile([128, G], F32R, name="x0r", tag="x0r")
                    nc.scalar.activation(
                        xr[:], xps[:], AF.Relu, bias=b_in_sb[:, m : m + 1]
                    )
                    x0_r.append(xr)

                for j in range(KC):
                    acts = []
                    for gi in range(4):
                        m = gi * KC + j
                        gps = gpsum.tile([128, G], F32, name="gps", tag="gps")
                        mms = [
                            (w_ih_r[:, ds(k * NGATE + m * 128, 128)], x0_r[k])
                            for k in range(KC)
                        ]
                        if not first:
                            mms += [
                                (w_hh_r[:, ds(k * NGATE + m * 128, 128)], h_prev[k])
                                for k in range(KC)
                            ]
                        for idx, (lhsT, rhs) in enumerate(mms):
                            nc.tensor.matmul(
                                gps[:],
                                lhsT,
                                rhs[:],
                                start=(idx == 0),
                                stop=(idx == len(mms) - 1),
                            )
                        a = act_pool.tile([128, G], F32, name="act", tag="act")
                        nc.scalar.activation(
                            a[:], gps[:], gate_funcs[gi], bias=b_g_sb[:, m : m + 1]
                        )
                        acts.append(a)
                    a_i, a_f, a_g, a_o = acts
                    if first:
                        nc.vector.tensor_mul(c_t[j][:], a_i[:], a_g[:])
                    else:
                        t1 = tmp_pool.tile([128, G], F32, name="t1", tag="t1")
                        nc.vector.tensor_mul(t1[:], a_f[:], c_t[j][:])
                        t2 = tmp_pool.tile([128, G], F32, name="t2", tag="t2")
                        nc.vector.tensor_mul(t2[:], a_i[:], a_g[:])
                        nc.vector.tensor_add(c_t[j][:], t1[:], t2[:])
                    tanc = tmp_pool.tile([128, G], F32, name="tanc", tag="tanc")
                    nc.scalar.activation(tanc[:], c_t[j][:], AF.Tanh)
                    nc.vector.tensor_mul(h_cur[j][:], a_o[:], tanc[:])

                yps = ypsum.tile([1, G], F32, name="yps", tag="yps")
                for k in range(KC):
                    nc.tensor.matmul(
                        yps[:],
                        w_out_r[:, k : k + 1],
                        h_cur[k][:],
                        start=(k == 0),
                        stop=(k == KC - 1),
                    )
                y_sb = y_pool.tile([1, G], F32, name="y_sb", tag="y_sb")
                nc.scalar.activation(y_sb[:], yps[:], AF.Identity, bias=b_out_sb[:, 0:1])
                nc.gpsimd.dma_start(y_d[ds(t, 1)], y_sb[:])

            step(0, first=True, h_prev=None, h_cur=h_a)
            # steps 1.. alternate: odd t reads h_a writes h_b, even t reads h_b
            # writes h_a  (t=0 wrote h_a)
            if use_loop:
                assert (nt - 1) % unroll == 0 and unroll % 2 == 0
                with tc.For_i(1, nt, unroll, staggered_reset=staggered) as iv:
                    for u in range(unroll):
                        hp, hc = (h_a, h_b) if u % 2 == 0 else (h_b, h_a)
                        step(iv + u, first=False, h_prev=hp, h_cur=hc)
            else:
                for t in range(1, nt):
                    hp, hc = (h_a, h_b) if t % 2 == 1 else (h_b, h_a)
                    step(t, first=False, h_prev=hp, h_cur=hc)

    nc.compile()
    return nc


def _prep_inputs(nt, inputs, W_in, b_in, w_ih, w_hh, b_ih, b_hh, W_out, b_out):
    f = np.float32
    inputs = np.ascontiguousarray(np.asarray(inputs, f))
    wihT = np.ascontiguousarray(np.asarray(w_ih, f).T)  # [HID, 4H]
    whhT = np.ascontiguousarray(np.asarray(w_hh, f).T)
    winT = np.ascontiguousarray(np.asarray(W_in, f).T)  # [NX, HID]
    woutC = np.ascontiguousarray(np.asarray(W_out, f).reshape(NY, KC, 128)[0].T)
    binC = np.ascontiguousarray(np.asarray(b_in, f).reshape(KC, 128).T)
    bgC = np.ascontiguousarray(
        (np.asarray(b_ih, f) + np.asarray(b_hh, f)).reshape(MC, 128).T
    )
    bout = np.asarray(b_out, f).reshape(1, 1)
    shared = dict(
        wihT=wihT, whhT=whhT, winT=winT, woutC=woutC, binC=binC, bgC=bgC, bout=bout
    )
    in_maps = []
    for c in range(N_CORES):
        xc = inputs[:nt, c * G : (c + 1) * G, :]  # [nt, G, NX]
        xt = np.ascontiguousarray(xc.transpose(0, 2, 1)).reshape(nt * NX, G)
        in_maps.append({"xt": xt, **shared})
    return in_maps


def run(inputs_dict, trace=False, nt=NT, unroll=U, use_loop=True, staggered=False, psum6=False, **spmd_kwargs):
    nc = build_program(nt, unroll, use_loop, staggered, psum6)
    in_maps = _prep_inputs(nt, **inputs_dict)
    res = run_bass_kernel_spmd(
        nc, in_maps, core_ids=list(range(N_CORES)), trace=trace, **spmd_kwargs
    )
    out = np.empty((nt, NGRID, NY), np.float32)
    for c in range(N_CORES):
        out[:, c * G : (c + 1) * G, 0] = res.results[c]["y"]
    return out, res


def kernel(**inputs):
    out, _ = run(inputs, trace=False)
    return out



# revision 4
# speedup vs baseline: 32.5978x; 32.5978x over previous
"""Trainium2 Bass kernel for CpuLstmModel (LSTM over basins).

Reference computation (per timestep t):
    x0    = relu(x_t @ W_in.T + b_in)                    # [G, H]
    gates = x0 @ w_ih.T + b_ih + h @ w_hh.T + b_hh       # [G, 4H]
    i,f,g,o = split(gates, 4)
    c = sigmoid(f)*c + sigmoid(i)*tanh(g)
    h = sigmoid(o)*tanh(c)
    y_t = h @ W_out.T + b_out                            # [G, 1]

Sharding: data-parallel over ngrid (4096 basins) across 8 cores; weights and
h/c state replicated. On-chip layout is feature-major ("transposed"):
activations live as [hid, basins] tiles so every matmul is
  out[gate_chunk(128), basins(512)] += W_chunk[k(128), gate_chunk(128)].T @ act[k(128), basins(512)]
with basins as the N=512 moving dim. Matmuls run in float32r.
"""

import numpy as np

import concourse.bass as bass
import concourse.mybir as mybir
import concourse.tile as tile
from concourse import bacc
from concourse.bass import ds, ts
from concourse.bass_utils import run_bass_kernel_spmd

NT = 365
NGRID = 4096
NX = 32
HID = 512
NY = 1
N_CORES = 8
G = NGRID // N_CORES  # basins per core = 512
KC = HID // 128  # hid chunks = 4
NGATE = 4 * HID  # 2048
MC = NGATE // 128  # gate chunks = 16

F32 = mybir.dt.float32
F32R = mybir.dt.float32r
AF = mybir.ActivationFunctionType

U = 28  # steps per hardware-loop iteration; must be EVEN (h ping-pong) and divide NT-1=364


def build_program(nt=NT, unroll=U, use_loop=True, staggered=False, psum6=False,
                  reps=1, bench_mode=False):
    nc = bacc.Bacc("TRN2", num_devices=N_CORES)

    xt_rows = NX if bench_mode else nt * NX
    xt_d = nc.dram_tensor("xt", [xt_rows, G], F32, kind="ExternalInput").ap()
    wih_d = nc.dram_tensor("wihT", [HID, NGATE], F32, kind="ExternalInput").ap()
    whh_d = nc.dram_tensor("whhT", [HID, NGATE], F32, kind="ExternalInput").ap()
    win_d = nc.dram_tensor("winT", [NX, HID], F32, kind="ExternalInput").ap()
    wout_d = nc.dram_tensor("woutC", [128, KC], F32, kind="ExternalInput").ap()
    bin_d = nc.dram_tensor("binC", [128, KC], F32, kind="ExternalInput").ap()
    bg_d = nc.dram_tensor("bgC", [128, MC], F32, kind="ExternalInput").ap()
    bout_d = nc.dram_tensor("bout", [1, 1], F32, kind="ExternalInput").ap()
    y_d = nc.dram_tensor("y", [nt, G], F32, kind="ExternalOutput").ap()

    with tile.TileContext(nc) as tc:
        with (
            tc.tile_pool(name="const", bufs=1) as cpool,
            tc.tile_pool(name="stag", bufs=2) as stag_pool,
            tc.tile_pool(name="xt", bufs=3) as xt_pool,
            tc.tile_pool(name="xtr", bufs=3) as xtr_pool,
            tc.tile_pool(name="x0", bufs=8) as x0_pool,
            tc.tile_pool(name="acts", bufs=10) as act_pool,
            tc.tile_pool(name="tmp", bufs=6) as tmp_pool,
            tc.tile_pool(name="state", bufs=1) as state_pool,
            tc.tile_pool(name="ysb", bufs=2) as y_pool,
            tc.tile_pool(name="gpsum", bufs=6 if psum6 else 5, space="PSUM") as gpsum,
            tc.tile_pool(name="xpsum", bufs=1 if psum6 else 2, space="PSUM") as xpsum,
            tc.tile_pool(name="ypsum", bufs=1, space="PSUM") as ypsum,
        ):
            # ---- load + round weights ----
            w_ih_r = cpool.tile([128, KC * NGATE], F32R, name="w_ih_r", tag="w_ih_r")
            w_hh_r = cpool.tile([128, KC * NGATE], F32R, name="w_hh_r", tag="w_hh_r")
            for k in range(KC):
                st = stag_pool.tile([128, NGATE], F32, name="st", tag="st")
                nc.gpsimd.dma_start(st[:], wih_d[ts(k, 128), :])
                nc.vector.tensor_copy(w_ih_r[:, ts(k, NGATE)], st[:])
                st2 = stag_pool.tile([128, NGATE], F32, name="st", tag="st")
                nc.gpsimd.dma_start(st2[:], whh_d[ts(k, 128), :])
                nc.vector.tensor_copy(w_hh_r[:, ts(k, NGATE)], st2[:])
            stw = stag_pool.tile([NX, HID], F32, name="stw", tag="stw")
            nc.gpsimd.dma_start(stw[:], win_d[:, :])
            w_in_r = cpool.tile([NX, HID], F32R, name="w_in_r", tag="w_in_r")
            nc.vector.tensor_copy(w_in_r[:], stw[:])
            sto = stag_pool.tile([128, KC], F32, name="sto", tag="sto")
            nc.gpsimd.dma_start(sto[:], wout_d[:, :])
            w_out_r = cpool.tile([128, KC], F32R, name="w_out_r", tag="w_out_r")
            nc.vector.tensor_copy(w_out_r[:], sto[:])
            b_in_sb = cpool.tile([128, KC], F32, name="b_in_sb", tag="b_in_sb")
            nc.gpsimd.dma_start(b_in_sb[:], bin_d[:, :])
            b_g_sb = cpool.tile([128, MC], F32, name="b_g_sb", tag="b_g_sb")
            nc.gpsimd.dma_start(b_g_sb[:], bg_d[:, :])
            b_out_sb = cpool.tile([1, 1], F32, name="b_out_sb", tag="b_out_sb")
            nc.gpsimd.dma_start(b_out_sb[:], bout_d[:, :])

            # ---- persistent state (h double-buffered: gate matmuls of step t
            # read h from step t-1 while the elementwise tail writes step t's) ----
            h_a = [state_pool.tile([128, G], F32R, name=f"ha{j}", tag=f"ha{j}") for j in range(KC)]
            h_b = [state_pool.tile([128, G], F32R, name=f"hb{j}", tag=f"hb{j}") for j in range(KC)]
            c_t = [state_pool.tile([128, G], F32, name=f"c{j}", tag=f"c{j}") for j in range(KC)]

            gate_funcs = [AF.Sigmoid, AF.Sigmoid, AF.Tanh, AF.Sigmoid]  # i, f, g, o

            def step(t, first, h_prev, h_cur):
                xt_sb = xt_pool.tile([NX, G], F32, name="xt_sb", tag="xt_sb")
                xt_src = xt_d[ds(0, NX), :] if bench_mode else xt_d[ts(t, NX), :]
                nc.gpsimd.dma_start(xt_sb[:], xt_src)
                xt_r = xtr_pool.tile([NX, G], F32R, name="xt_r", tag="xt_r")
                nc.vector.tensor_copy(xt_r[:], xt_sb[:])

                x0_r = []
                for m in range(KC):
                    xps = xpsum.tile([128, G], F32, name="xps", tag="xps")
                    nc.tensor.matmul(
                        xps[:], w_in_r[:, ts(m, 128)], xt_r[:], start=True, stop=True
                    )
                    xr = x0_pool.tile([128, G], F32R, name="x0r", tag="x0r")
                    nc.scalar.activation(
                        xr[:], xps[:], AF.Relu, bias=b_in_sb[:, m : m + 1]
                    )
                    x0_r.append(xr)

                for j in range(KC):
                    acts = []
                    for gi in range(4):
                        m = gi * KC + j
                        gps = gpsum.tile([128, G], F32, name="gps", tag="gps")
                        mms = [
                            (w_ih_r[:, ds(k * NGATE + m * 128, 128)], x0_r[k])
                            for k in range(KC)
                        ]
                        if not first:
                            mms += [
                                (w_hh_r[:, ds(k * NGATE + m * 128, 128)], h_prev[k])
                                for k in range(KC)
                            ]
                        for idx, (lhsT, rhs) in enumerate(mms):
                            nc.tensor.matmul(
                                gps[:],
                                lhsT,
                                rhs[:],
                                start=(idx == 0),
                                stop=(idx == len(mms) - 1),
                            )
                        a = act_pool.tile([128, G], F32, name="act", tag="act")
                        nc.scalar.activation(
                            a[:], gps[:], gate_funcs[gi], bias=b_g_sb[:, m : m + 1]
                        )
                        acts.append(a)
                    a_i, a_f, a_g, a_o = acts
                    if first:
                        nc.vector.tensor_mul(c_t[j][:], a_i[:], a_g[:])
                    else:
                        t1 = tmp_pool.tile([128, G], F32, name="t1", tag="t1")
                        nc.vector.tensor_mul(t1[:], a_f[:], c_t[j][:])
                        t2 = tmp_pool.tile([128, G], F32, name="t2", tag="t2")
                        nc.vector.tensor_mul(t2[:], a_i[:], a_g[:])
                        nc.vector.tensor_add(c_t[j][:], t1[:], t2[:])
                    tanc = tmp_pool.tile([128, G], F32, name="tanc", tag="tanc")
                    nc.scalar.activation(tanc[:], c_t[j][:], AF.Tanh)
                    nc.vector.tensor_mul(h_cur[j][:], a_o[:], tanc[:])

                yps = ypsum.tile([1, G], F32, name="yps", tag="yps")
                for k in range(KC):
                    nc.tensor.matmul(
                        yps[:],
                        w_out_r[:, k : k + 1],
                        h_cur[k][:],
                        start=(k == 0),
                        stop=(k == KC - 1),
                    )
                y_sb = y_pool.tile([1, G], F32, name="y_sb", tag="y_sb")
                nc.scalar.activation(y_sb[:], yps[:], AF.Identity, bias=b_out_sb[:, 0:1])
                nc.gpsimd.dma_start(y_d[ds(t, 1)], y_sb[:])

            def one_pass():
                step(0, first=True, h_prev=None, h_cur=h_a)
                # steps 1.. alternate: odd t reads h_a writes h_b, even t
                # reads h_b writes h_a  (t=0 wrote h_a)
                if use_loop:
                    assert (nt - 1) % unroll == 0 and unroll % 2 == 0
                    with tc.For_i(1, nt, unroll, staggered_reset=staggered) as iv:
                        for u in range(unroll):
                            hp, hc = (h_a, h_b) if u % 2 == 0 else (h_b, h_a)
                            step(iv + u, first=False, h_prev=hp, h_cur=hc)
                else:
                    for t in range(1, nt):
                        hp, hc = (h_a, h_b) if t % 2 == 1 else (h_b, h_a)
                        step(t, first=False, h_prev=hp, h_cur=hc)

            if reps == 1:
                one_pass()
            else:
                with tc.For_i(0, reps, 1):
                    one_pass()

    nc.compile()
    return nc


def _prep_inputs(nt, inputs, W_in, b_in, w_ih, w_hh, b_ih, b_hh, W_out, b_out):
    f = np.float32
    inputs = np.ascontiguousarray(np.asarray(inputs, f))
    wihT = np.ascontiguousarray(np.asarray(w_ih, f).T)  # [HID, 4H]
    whhT = np.ascontiguousarray(np.asarray(w_hh, f).T)
    winT = np.ascontiguousarray(np.asarray(W_in, f).T)  # [NX, HID]
    woutC = np.ascontiguousarray(np.asarray(W_out, f).reshape(NY, KC, 128)[0].T)
    binC = np.ascontiguousarray(np.asarray(b_in, f).reshape(KC, 128).T)
    bgC = np.ascontiguousarray(
        (np.asarray(b_ih, f) + np.asarray(b_hh, f)).reshape(MC, 128).T
    )
    bout = np.asarray(b_out, f).reshape(1, 1)
    shared = dict(
        wihT=wihT, whhT=whhT, winT=winT, woutC=woutC, binC=binC, bgC=bgC, bout=bout
    )
    in_maps = []
    for c in range(N_CORES):
        xc = inputs[:nt, c * G : (c + 1) * G, :]  # [nt, G, NX]
        xt = np.ascontiguousarray(xc.transpose(0, 2, 1)).reshape(nt * NX, G)
        in_maps.append({"xt": xt, **shared})
    return in_maps


def run(inputs_dict, trace=False, nt=NT, unroll=U, use_loop=True, staggered=False, psum6=False, **spmd_kwargs):
    nc = build_program(nt, unroll, use_loop, staggered, psum6)
    in_maps = _prep_inputs(nt, **inputs_dict)
    res = run_bass_kernel_spmd(
        nc, in_maps, core_ids=list(range(N_CORES)), trace=trace, **spmd_kwargs
    )
    out = np.empty((nt, NGRID, NY), np.float32)
    for c in range(N_CORES):
        out[:, c * G : (c + 1) * G, 0] = res.results[c]["y"]
    return out, res


def kernel(**inputs):
    out, _ = run(inputs, trace=False)
    return out



# revision 8
# speedup vs baseline: 33.5968x; 1.0306x over previous
"""Trainium2 Bass kernel for CpuLstmModel (LSTM over basins).

Reference computation (per timestep t):
    x0    = relu(x_t @ W_in.T + b_in)                    # [G, H]
    gates = x0 @ w_ih.T + b_ih + h @ w_hh.T + b_hh       # [G, 4H]
    i,f,g,o = split(gates, 4)
    c = sigmoid(f)*c + sigmoid(i)*tanh(g)
    h = sigmoid(o)*tanh(c)
    y_t = h @ W_out.T + b_out                            # [G, 1]

Sharding: data-parallel over ngrid (4096 basins) across 8 cores; weights and
h/c state replicated. On-chip layout is feature-major ("transposed"):
activations live as [hid, basins] tiles so every matmul is
  out[gate_chunk(128), basins(512)] += W_chunk[k(128), gate_chunk(128)].T @ act[k(128), basins(512)]
with basins as the N=512 moving dim. Matmuls run in bf16 (weights, x0, h);
accumulation and the c state stay fp32.
"""

import numpy as np

import concourse.bass as bass
import concourse.mybir as mybir
import concourse.tile as tile
from concourse import bacc
from concourse.bass import ds, ts
from concourse.bass_utils import run_bass_kernel_spmd

NT = 365
NGRID = 4096
NX = 32
HID = 512
NY = 1
N_CORES = 8
G = NGRID // N_CORES  # basins per core = 512
KC = HID // 128  # hid chunks = 4
NGATE = 4 * HID  # 2048
MC = NGATE // 128  # gate chunks = 16

F32 = mybir.dt.float32
F32R = mybir.dt.float32r
AF = mybir.ActivationFunctionType

U = 28  # steps per hardware-loop iteration; must be EVEN (h ping-pong) and divide NT-1=364


def build_program(nt=NT, unroll=U, use_loop=True, staggered=False, psum6=False,
                  reps=1, bench_mode=False, shared_psum=True, bf16=True):
    nc = bacc.Bacc("TRN2", num_devices=N_CORES)

    xt_rows = NX if bench_mode else nt * NX
    xt_d = nc.dram_tensor("xt", [xt_rows, G], F32, kind="ExternalInput").ap()
    wih_d = nc.dram_tensor("wihT", [HID, NGATE], F32, kind="ExternalInput").ap()
    whh_d = nc.dram_tensor("whhT", [HID, NGATE], F32, kind="ExternalInput").ap()
    win_d = nc.dram_tensor("winT", [NX, HID], F32, kind="ExternalInput").ap()
    wout_d = nc.dram_tensor("woutC", [128, KC], F32, kind="ExternalInput").ap()
    bin_d = nc.dram_tensor("binC", [128, KC], F32, kind="ExternalInput").ap()
    bg_d = nc.dram_tensor("bgC", [128, MC], F32, kind="ExternalInput").ap()
    bout_d = nc.dram_tensor("bout", [1, 1], F32, kind="ExternalInput").ap()
    y_d = nc.dram_tensor("y", [nt, G], F32, kind="ExternalOutput").ap()

    WDT = mybir.dt.bfloat16 if bf16 else F32R
    from contextlib import ExitStack

    with tile.TileContext(nc) as tc, ExitStack() as ctx:
        cpool = ctx.enter_context(tc.tile_pool(name="const", bufs=1))
        stag_pool = ctx.enter_context(tc.tile_pool(name="stag", bufs=2))
        xt_pool = ctx.enter_context(tc.tile_pool(name="xt", bufs=3))
        xtr_pool = ctx.enter_context(tc.tile_pool(name="xtr", bufs=3))
        x0_pool = ctx.enter_context(tc.tile_pool(name="x0", bufs=8))
        act_pool = ctx.enter_context(tc.tile_pool(name="acts", bufs=10))
        tmp_pool = ctx.enter_context(tc.tile_pool(name="tmp", bufs=6))
        state_pool = ctx.enter_context(tc.tile_pool(name="state", bufs=1))
        y_pool = ctx.enter_context(tc.tile_pool(name="ysb", bufs=2))
        if shared_psum:
            pspool = ctx.enter_context(
                tc.tile_pool(name="pspool", bufs=8, space="PSUM"))
            gpsum = xpsum = ypsum = pspool
            g_tag = x_tag = y_tag = "ps"
        else:
            gpsum = ctx.enter_context(tc.tile_pool(
                name="gpsum", bufs=6 if psum6 else 5, space="PSUM"))
            xpsum = ctx.enter_context(tc.tile_pool(
                name="xpsum", bufs=1 if psum6 else 2, space="PSUM"))
            ypsum = ctx.enter_context(tc.tile_pool(
                name="ypsum", bufs=1, space="PSUM"))
            g_tag, x_tag, y_tag = "gps", "xps", "yps"
        if True:
            # ---- load + round weights ----
            w_ih_r = cpool.tile([128, KC * NGATE], WDT, name="w_ih_r", tag="w_ih_r")
            w_hh_r = cpool.tile([128, KC * NGATE], WDT, name="w_hh_r", tag="w_hh_r")
            for k in range(KC):
                st = stag_pool.tile([128, NGATE], F32, name="st", tag="st")
                nc.gpsimd.dma_start(st[:], wih_d[ts(k, 128), :])
                nc.vector.tensor_copy(w_ih_r[:, ts(k, NGATE)], st[:])
                st2 = stag_pool.tile([128, NGATE], F32, name="st", tag="st")
                nc.gpsimd.dma_start(st2[:], whh_d[ts(k, 128), :])
                nc.vector.tensor_copy(w_hh_r[:, ts(k, NGATE)], st2[:])
            stw = stag_pool.tile([NX, HID], F32, name="stw", tag="stw")
            nc.gpsimd.dma_start(stw[:], win_d[:, :])
            w_in_r = cpool.tile([NX, HID], WDT, name="w_in_r", tag="w_in_r")
            nc.vector.tensor_copy(w_in_r[:], stw[:])
            sto = stag_pool.tile([128, KC], F32, name="sto", tag="sto")
            nc.gpsimd.dma_start(sto[:], wout_d[:, :])
            w_out_r = cpool.tile([128, KC], WDT, name="w_out_r", tag="w_out_r")
            nc.vector.tensor_copy(w_out_r[:], sto[:])
            b_in_sb = cpool.tile([128, KC], F32, name="b_in_sb", tag="b_in_sb")
            nc.gpsimd.dma_start(b_in_sb[:], bin_d[:, :])
            b_g_sb = cpool.tile([128, MC], F32, name="b_g_sb", tag="b_g_sb")
            nc.gpsimd.dma_start(b_g_sb[:], bg_d[:, :])
            b_out_sb = cpool.tile([1, 1], F32, name="b_out_sb", tag="b_out_sb")
            nc.gpsimd.dma_start(b_out_sb[:], bout_d[:, :])

            # ---- persistent state (h double-buffered: gate matmuls of step t
            # read h from step t-1 while the elementwise tail writes step t's) ----
            h_a = [state_pool.tile([128, G], WDT, name=f"ha{j}", tag=f"ha{j}") for j in range(KC)]
            h_b = [state_pool.tile([128, G], WDT, name=f"hb{j}", tag=f"hb{j}") for j in range(KC)]
            c_t = [state_pool.tile([128, G], F32, name=f"c{j}", tag=f"c{j}") for j in range(KC)]

            gate_funcs = [AF.Sigmoid, AF.Sigmoid, AF.Tanh, AF.Sigmoid]  # i, f, g, o

            def step(t, first, h_prev, h_cur):
                xt_sb = xt_pool.tile([NX, G], F32, name="xt_sb", tag="xt_sb")
                xt_src = xt_d[ds(0, NX), :] if bench_mode else xt_d[ts(t, NX), :]
                nc.gpsimd.dma_start(xt_sb[:], xt_src)
                xt_r = xtr_pool.tile([NX, G], WDT, name="xt_r", tag="xt_r")
                nc.vector.tensor_copy(xt_r[:], xt_sb[:])

                x0_r = []
                for m in range(KC):
                    xps = xpsum.tile([128, G], F32, name="xps", tag=x_tag)
                    nc.tensor.matmul(
                        xps[:], w_in_r[:, ts(m, 128)], xt_r[:], start=True, stop=True
                    )
                    xr = x0_pool.tile([128, G], WDT, name="x0r", tag="x0r")
                    nc.scalar.activation(
                        xr[:], xps[:], AF.Relu, bias=b_in_sb[:, m : m + 1]
                    )
                    x0_r.append(xr)

                for j in range(KC):
                    acts = []
                    for gi in range(4):
                        m = gi * KC + j
                        gps = gpsum.tile([128, G], F32, name="gps", tag=g_tag)
                        mms = [
                            (w_ih_r[:, ds(k * NGATE + m * 128, 128)], x0_r[k])
                            for k in range(KC)
                        ]
                        if not first:
                            mms += [
                                (w_hh_r[:, ds(k * NGATE + m * 128, 128)], h_prev[k])
                                for k in range(KC)
                            ]
                        for idx, (lhsT, rhs) in enumerate(mms):
                            nc.tensor.matmul(
                                gps[:],
                                lhsT,
                                rhs[:],
                                start=(idx == 0),
                                stop=(idx == len(mms) - 1),
                            )
                        a = act_pool.tile([128, G], F32, name="act", tag="act")
                        nc.scalar.activation(
                            a[:], gps[:], gate_funcs[gi], bias=b_g_sb[:, m : m + 1]
                        )
                        acts.append(a)
                    a_i, a_f, a_g, a_o = acts
                    if first:
                        nc.vector.tensor_mul(c_t[j][:], a_i[:], a_g[:])
                    else:
                        t1 = tmp_pool.tile([128, G], F32, name="t1", tag="t1")
                        nc.vector.tensor_mul(t1[:], a_f[:], c_t[j][:])
                        t2 = tmp_pool.tile([128, G], F32, name="t2", tag="t2")
                        nc.vector.tensor_mul(t2[:], a_i[:], a_g[:])
                        nc.vector.tensor_add(c_t[j][:], t1[:], t2[:])
                    tanc = tmp_pool.tile([128, G], F32, name="tanc", tag="tanc")
                    nc.scalar.activation(tanc[:], c_t[j][:], AF.Tanh)
                    nc.vector.tensor_mul(h_cur[j][:], a_o[:], tanc[:])

                yps = ypsum.tile([1, G], F32, name="yps", tag=y_tag)
                for k in range(KC):
                    nc.tensor.matmul(
                        yps[:],
                        w_out_r[:, k : k + 1],
                        h_cur[k][:],
                        start=(k == 0),
                        stop=(k == KC - 1),
                    )
                y_sb = y_pool.tile([1, G], F32, name="y_sb", tag="y_sb")
                nc.scalar.activation(y_sb[:], yps[:], AF.Identity, bias=b_out_sb[:, 0:1])
                nc.gpsimd.dma_start(y_d[ds(t, 1)], y_sb[:])

            def one_pass():
                step(0, first=True, h_prev=None, h_cur=h_a)
                # steps 1.. alternate: odd t reads h_a writes h_b, even t
                # reads h_b writes h_a  (t=0 wrote h_a)
                if use_loop:
                    assert (nt - 1) % unroll == 0 and unroll % 2 == 0
                    with tc.For_i(1, nt, unroll, staggered_reset=staggered) as iv:
                        for u in range(unroll):
                            hp, hc = (h_a, h_b) if u % 2 == 0 else (h_b, h_a)
                            step(iv + u, first=False, h_prev=hp, h_cur=hc)
                else:
                    for t in range(1, nt):
                        hp, hc = (h_a, h_b) if t % 2 == 1 else (h_b, h_a)
                        step(t, first=False, h_prev=hp, h_cur=hc)

            if reps == 1:
                one_pass()
            else:
                with tc.For_i(0, reps, 1):
                    one_pass()

    nc.compile()
    return nc


def _prep_inputs(nt, inputs, W_in, b_in, w_ih, w_hh, b_ih, b_hh, W_out, b_out):
    f = np.float32
    inputs = np.ascontiguousarray(np.asarray(inputs, f))
    wihT = np.ascontiguousarray(np.asarray(w_ih, f).T)  # [HID, 4H]
    whhT = np.ascontiguousarray(np.asarray(w_hh, f).T)
    winT = np.ascontiguousarray(np.asarray(W_in, f).T)  # [NX, HID]
    woutC = np.ascontiguousarray(np.asarray(W_out, f).reshape(NY, KC, 128)[0].T)
    binC = np.ascontiguousarray(np.asarray(b_in, f).reshape(KC, 128).T)
    bgC = np.ascontiguousarray(
        (np.asarray(b_ih, f) + np.asarray(b_hh, f)).reshape(MC, 128).T
    )
    bout = np.asarray(b_out, f).reshape(1, 1)
    shared = dict(
        wihT=wihT, whhT=whhT, winT=winT, woutC=woutC, binC=binC, bgC=bgC, bout=bout
    )
    in_maps = []
    for c in range(N_CORES):
        xc = inputs[:nt, c * G : (c + 1) * G, :]  # [nt, G, NX]
        xt = np.ascontiguousarray(xc.transpose(0, 2, 1)).reshape(nt * NX, G)
        in_maps.append({"xt": xt, **shared})
    return in_maps


def run(inputs_dict, trace=False, nt=NT, unroll=U, use_loop=True, staggered=False, psum6=False, **spmd_kwargs):
    nc = build_program(nt, unroll, use_loop, staggered, psum6)
    in_maps = _prep_inputs(nt, **inputs_dict)
    res = run_bass_kernel_spmd(
        nc, in_maps, core_ids=list(range(N_CORES)), trace=trace, **spmd_kwargs
    )
    out = np.empty((nt, NGRID, NY), np.float32)
    for c in range(N_CORES):
        out[:, c * G : (c + 1) * G, 0] = res.results[c]["y"]
    return out, res


def kernel(**inputs):
    out, _ = run(inputs, trace=False)
    return out



# revision 10
# speedup vs baseline: 34.3594x; 1.0227x over previous
"""Trainium2 Bass kernel for CpuLstmModel (LSTM over basins).

Reference computation (per timestep t):
    x0    = relu(x_t @ W_in.T + b_in)                    # [G, H]
    gates = x0 @ w_ih.T + b_ih + h @ w_hh.T + b_hh       # [G, 4H]
    i,f,g,o = split(gates, 4)
    c = sigmoid(f)*c + sigmoid(i)*tanh(g)
    h = sigmoid(o)*tanh(c)
    y_t = h @ W_out.T + b_out                            # [G, 1]

Sharding: data-parallel over ngrid (4096 basins) across 8 cores; weights and
h/c state replicated. On-chip layout is feature-major ("transposed"):
activations live as [hid, basins] tiles so every matmul is
  out[gate_chunk(128), basins(512)] += W_chunk[k(128), gate_chunk(128)].T @ act[k(128), basins(512)]
with basins as the N=512 moving dim. Matmuls run in bf16 (weights, x0, h);
accumulation and the c state stay fp32.
"""

import numpy as np

import concourse.bass as bass
import concourse.mybir as mybir
import concourse.tile as tile
from concourse import bacc
from concourse.bass import ds, ts
from concourse.bass_utils import run_bass_kernel_spmd

NT = 365
NGRID = 4096
NX = 32
HID = 512
NY = 1
N_CORES = 8
G = NGRID // N_CORES  # basins per core = 512
KC = HID // 128  # hid chunks = 4
NGATE = 4 * HID  # 2048
MC = NGATE // 128  # gate chunks = 16

F32 = mybir.dt.float32
F32R = mybir.dt.float32r
AF = mybir.ActivationFunctionType

U = 28  # steps per hardware-loop iteration; must be EVEN (h ping-pong) and divide NT-1=364


def build_program(nt=NT, unroll=U, use_loop=True, staggered=False, psum6=False,
                  reps=1, bench_mode=False, shared_psum=True, bf16=True):
    nc = bacc.Bacc("TRN2", num_devices=N_CORES)

    xt_rows = NX if bench_mode else nt * NX
    xt_d = nc.dram_tensor("xt", [xt_rows, G], F32, kind="ExternalInput").ap()
    wih_d = nc.dram_tensor("wihT", [HID, NGATE], F32, kind="ExternalInput").ap()
    whh_d = nc.dram_tensor("whhT", [HID, NGATE], F32, kind="ExternalInput").ap()
    win_d = nc.dram_tensor("winT", [NX, HID], F32, kind="ExternalInput").ap()
    wout_d = nc.dram_tensor("woutC", [128, KC], F32, kind="ExternalInput").ap()
    bin_d = nc.dram_tensor("binC", [128, KC], F32, kind="ExternalInput").ap()
    bg_d = nc.dram_tensor("bgC", [128, MC], F32, kind="ExternalInput").ap()
    bout_d = nc.dram_tensor("bout", [1, 1], F32, kind="ExternalInput").ap()
    y_d = nc.dram_tensor("y", [nt, G], F32, kind="ExternalOutput").ap()

    WDT = mybir.dt.bfloat16 if bf16 else F32R
    from contextlib import ExitStack

    with tile.TileContext(nc) as tc, ExitStack() as ctx:
        cpool = ctx.enter_context(tc.tile_pool(name="const", bufs=1))
        stag_pool = ctx.enter_context(tc.tile_pool(name="stag", bufs=2))
        xt_pool = ctx.enter_context(tc.tile_pool(name="xt", bufs=3))
        xtr_pool = ctx.enter_context(tc.tile_pool(name="xtr", bufs=3))
        x0_pool = ctx.enter_context(tc.tile_pool(name="x0", bufs=8))
        act_pool = ctx.enter_context(tc.tile_pool(name="acts", bufs=10))
        tmp_pool = ctx.enter_context(tc.tile_pool(name="tmp", bufs=6))
        state_pool = ctx.enter_context(tc.tile_pool(name="state", bufs=1))
        y_pool = ctx.enter_context(tc.tile_pool(name="ysb", bufs=2))
        if shared_psum:
            pspool = ctx.enter_context(
                tc.tile_pool(name="pspool", bufs=8, space="PSUM"))
            gpsum = xpsum = ypsum = pspool
            g_tag = x_tag = y_tag = "ps"
        else:
            gpsum = ctx.enter_context(tc.tile_pool(
                name="gpsum", bufs=6 if psum6 else 5, space="PSUM"))
            xpsum = ctx.enter_context(tc.tile_pool(
                name="xpsum", bufs=1 if psum6 else 2, space="PSUM"))
            ypsum = ctx.enter_context(tc.tile_pool(
                name="ypsum", bufs=1, space="PSUM"))
            g_tag, x_tag, y_tag = "gps", "xps", "yps"
        if True:
            # ---- load + round weights ----
            w_ih_r = cpool.tile([128, KC * NGATE], WDT, name="w_ih_r", tag="w_ih_r")
            w_hh_r = cpool.tile([128, KC * NGATE], WDT, name="w_hh_r", tag="w_hh_r")
            for k in range(KC):
                st = stag_pool.tile([128, NGATE], F32, name="st", tag="st")
                nc.gpsimd.dma_start(st[:], wih_d[ts(k, 128), :])
                nc.vector.tensor_copy(w_ih_r[:, ts(k, NGATE)], st[:])
                st2 = stag_pool.tile([128, NGATE], F32, name="st", tag="st")
                nc.gpsimd.dma_start(st2[:], whh_d[ts(k, 128), :])
                nc.vector.tensor_copy(w_hh_r[:, ts(k, NGATE)], st2[:])
            stw = stag_pool.tile([NX, HID], F32, name="stw", tag="stw")
            nc.gpsimd.dma_start(stw[:], win_d[:, :])
            w_in_r = cpool.tile([NX, HID], WDT, name="w_in_r", tag="w_in_r")
            nc.vector.tensor_copy(w_in_r[:], stw[:])
            sto = stag_pool.tile([128, KC], F32, name="sto", tag="sto")
            nc.gpsimd.dma_start(sto[:], wout_d[:, :])
            w_out_r = cpool.tile([128, KC], WDT, name="w_out_r", tag="w_out_r")
            nc.vector.tensor_copy(w_out_r[:], sto[:])
            b_in_sb = cpool.tile([128, KC], F32, name="b_in_sb", tag="b_in_sb")
            nc.gpsimd.dma_start(b_in_sb[:], bin_d[:, :])
            b_g_sb = cpool.tile([128, MC], F32, name="b_g_sb", tag="b_g_sb")
            nc.gpsimd.dma_start(b_g_sb[:], bg_d[:, :])
            b_out_sb = cpool.tile([1, 1], F32, name="b_out_sb", tag="b_out_sb")
            nc.gpsimd.dma_start(b_out_sb[:], bout_d[:, :])

            # ---- persistent state (h double-buffered: gate matmuls of step t
            # read h from step t-1 while the elementwise tail writes step t's) ----
            h_a = [state_pool.tile([128, G], WDT, name=f"ha{j}", tag=f"ha{j}") for j in range(KC)]
            h_b = [state_pool.tile([128, G], WDT, name=f"hb{j}", tag=f"hb{j}") for j in range(KC)]
            c_t = [state_pool.tile([128, G], F32, name=f"c{j}", tag=f"c{j}") for j in range(KC)]

            gate_funcs = [AF.Sigmoid, AF.Sigmoid, AF.Tanh, AF.Sigmoid]  # i, f, g, o

            def emit_y(t_out, h_src):
                # y_{t_out} = h_{t_out} @ W_out + b_out, computed from h_src
                yps = ypsum.tile([1, G], F32, name="yps", tag=y_tag)
                for k in range(KC):
                    nc.tensor.matmul(
                        yps[:],
                        w_out_r[:, k : k + 1],
                        h_src[k][:],
                        start=(k == 0),
                        stop=(k == KC - 1),
                    )
                y_sb = y_pool.tile([1, G], F32, name="y_sb", tag="y_sb")
                nc.scalar.activation(y_sb[:], yps[:], AF.Identity, bias=b_out_sb[:, 0:1])
                nc.gpsimd.dma_start(y_d[ds(t_out, 1)] if isinstance(t_out, int) else y_d[ds(t_out, 1)], y_sb[:])

            def step(t, first, h_prev, h_cur):
                if not first:
                    # y_{t-1} from h_prev: identical values, but schedulable
                    # early instead of extending step t-1's serial tail.
                    emit_y(t - 1, h_prev)
                xt_sb = xt_pool.tile([NX, G], F32, name="xt_sb", tag="xt_sb")
                xt_src = xt_d[ds(0, NX), :] if bench_mode else xt_d[ts(t, NX), :]
                nc.gpsimd.dma_start(xt_sb[:], xt_src)
                xt_r = xtr_pool.tile([NX, G], WDT, name="xt_r", tag="xt_r")
                nc.vector.tensor_copy(xt_r[:], xt_sb[:])

                x0_r = []
                for m in range(KC):
                    xps = xpsum.tile([128, G], F32, name="xps", tag=x_tag)
                    nc.tensor.matmul(
                        xps[:], w_in_r[:, ts(m, 128)], xt_r[:], start=True, stop=True
                    )
                    xr = x0_pool.tile([128, G], WDT, name="x0r", tag="x0r")
                    nc.scalar.activation(
                        xr[:], xps[:], AF.Relu, bias=b_in_sb[:, m : m + 1]
                    )
                    x0_r.append(xr)

                for j in range(KC):
                    acts = []
                    for gi in range(4):
                        m = gi * KC + j
                        gps = gpsum.tile([128, G], F32, name="gps", tag=g_tag)
                        mms = [
                            (w_ih_r[:, ds(k * NGATE + m * 128, 128)], x0_r[k])
                            for k in range(KC)
                        ]
                        if not first:
                            mms += [
                                (w_hh_r[:, ds(k * NGATE + m * 128, 128)], h_prev[k])
                                for k in range(KC)
                            ]
                        for idx, (lhsT, rhs) in enumerate(mms):
                            nc.tensor.matmul(
                                gps[:],
                                lhsT,
                                rhs[:],
                                start=(idx == 0),
                                stop=(idx == len(mms) - 1),
                            )
                        a = act_pool.tile([128, G], F32, name="act", tag="act")
                        nc.scalar.activation(
                            a[:], gps[:], gate_funcs[gi], bias=b_g_sb[:, m : m + 1]
                        )
                        acts.append(a)
                    a_i, a_f, a_g, a_o = acts
                    if first:
                        nc.vector.tensor_mul(c_t[j][:], a_i[:], a_g[:])
                    else:
                        t1 = tmp_pool.tile([128, G], F32, name="t1", tag="t1")
                        nc.vector.tensor_mul(t1[:], a_f[:], c_t[j][:])
                        t2 = tmp_pool.tile([128, G], F32, name="t2", tag="t2")
                        nc.vector.tensor_mul(t2[:], a_i[:], a_g[:])
                        nc.vector.tensor_add(c_t[j][:], t1[:], t2[:])
                    tanc = tmp_pool.tile([128, G], F32, name="tanc", tag="tanc")
                    nc.scalar.activation(tanc[:], c_t[j][:], AF.Tanh)
                    nc.vector.tensor_mul(h_cur[j][:], a_o[:], tanc[:])


            def one_pass():
                step(0, first=True, h_prev=None, h_cur=h_a)
                # steps 1.. alternate: odd t reads h_a writes h_b, even t
                # reads h_b writes h_a  (t=0 wrote h_a)
                if use_loop:
                    assert (nt - 1) % unroll == 0 and unroll % 2 == 0
                    with tc.For_i(1, nt, unroll, staggered_reset=staggered) as iv:
                        for u in range(unroll):
                            hp, hc = (h_a, h_b) if u % 2 == 0 else (h_b, h_a)
                            step(iv + u, first=False, h_prev=hp, h_cur=hc)
                else:
                    for t in range(1, nt):
                        hp, hc = (h_a, h_b) if t % 2 == 1 else (h_b, h_a)
                        step(t, first=False, h_prev=hp, h_cur=hc)
                # last step's y ((nt-1) even -> h in h_a)
                emit_y(nt - 1, h_a if (nt - 1) % 2 == 0 else h_b)

            if reps == 1:
                one_pass()
            else:
                with tc.For_i(0, reps, 1):
                    one_pass()

    nc.compile()
    return nc


def _prep_inputs(nt, inputs, W_in, b_in, w_ih, w_hh, b_ih, b_hh, W_out, b_out):
    f = np.float32
    inputs = np.ascontiguousarray(np.asarray(inputs, f))
    wihT = np.ascontiguousarray(np.asarray(w_ih, f).T)  # [HID, 4H]
    whhT = np.ascontiguousarray(np.asarray(w_hh, f).T)
    winT = np.ascontiguousarray(np.asarray(W_in, f).T)  # [NX, HID]
    woutC = np.ascontiguousarray(np.asarray(W_out, f).reshape(NY, KC, 128)[0].T)
    binC = np.ascontiguousarray(np.asarray(b_in, f).reshape(KC, 128).T)
    bgC = np.ascontiguousarray(
        (np.asarray(b_ih, f) + np.asarray(b_hh, f)).reshape(MC, 128).T
    )
    bout = np.asarray(b_out, f).reshape(1, 1)
    shared = dict(
        wihT=wihT, whhT=whhT, winT=winT, woutC=woutC, binC=binC, bgC=bgC, bout=bout
    )
    in_maps = []
    for c in range(N_CORES):
        xc = inputs[:nt, c * G : (c + 1) * G, :]  # [nt, G, NX]
        xt = np.ascontiguousarray(xc.transpose(0, 2, 1)).reshape(nt * NX, G)
        in_maps.append({"xt": xt, **shared})
    return in_maps


def run(inputs_dict, trace=False, nt=NT, unroll=U, use_loop=True, staggered=False, psum6=False, **spmd_kwargs):
    nc = build_program(nt, unroll, use_loop, staggered, psum6)
    in_maps = _prep_inputs(nt, **inputs_dict)
    res = run_bass_kernel_spmd(
        nc, in_maps, core_ids=list(range(N_CORES)), trace=trace, **spmd_kwargs
    )
    out = np.empty((nt, NGRID, NY), np.float32)
    for c in range(N_CORES):
        out[:, c * G : (c + 1) * G, 0] = res.results[c]["y"]
    return out, res


def kernel(**inputs):
    out, _ = run(inputs, trace=False)
    return out



# revision 11
# speedup vs baseline: 36.5434x; 1.0636x over previous
"""Trainium2 Bass kernel for CpuLstmModel (LSTM over basins).

Reference computation (per timestep t):
    x0    = relu(x_t @ W_in.T + b_in)                    # [G, H]
    gates = x0 @ w_ih.T + b_ih + h @ w_hh.T + b_hh       # [G, 4H]
    i,f,g,o = split(gates, 4)
    c = sigmoid(f)*c + sigmoid(i)*tanh(g)
    h = sigmoid(o)*tanh(c)
    y_t = h @ W_out.T + b_out                            # [G, 1]

Sharding: data-parallel over ngrid (4096 basins) across 8 cores; weights and
h/c state replicated. On-chip layout is feature-major ("transposed"):
activations live as [hid, basins] tiles so every matmul is
  out[gate_chunk(128), basins(512)] += W_chunk[k(128), gate_chunk(128)].T @ act[k(128), basins(512)]
with basins as the N=512 moving dim. Matmuls run in bf16 (weights, x0, h);
accumulation and the c state stay fp32.
"""

import numpy as np

import concourse.bass as bass
import concourse.mybir as mybir
import concourse.tile as tile
from concourse import bacc
from concourse.bass import ds, ts
from concourse.bass_utils import run_bass_kernel_spmd

NT = 365
NGRID = 4096
NX = 32
HID = 512
NY = 1
N_CORES = 8
G = NGRID // N_CORES  # basins per core = 512
KC = HID // 128  # hid chunks = 4
NGATE = 4 * HID  # 2048
MC = NGATE // 128  # gate chunks = 16

F32 = mybir.dt.float32
F32R = mybir.dt.float32r
AF = mybir.ActivationFunctionType

U = 28  # steps per hardware-loop iteration; must be EVEN (h ping-pong) and divide NT-1=364


def build_program(nt=NT, unroll=U, use_loop=True, staggered=False, psum6=False,
                  reps=1, bench_mode=False, shared_psum=True, bf16=True):
    nc = bacc.Bacc("TRN2", num_devices=N_CORES)

    xt_rows = NX if bench_mode else nt * NX
    XDT = mybir.dt.bfloat16 if bf16 else F32
    xt_d = nc.dram_tensor("xt", [xt_rows, G], XDT, kind="ExternalInput").ap()
    wih_d = nc.dram_tensor("wihT", [HID, NGATE], F32, kind="ExternalInput").ap()
    whh_d = nc.dram_tensor("whhT", [HID, NGATE], F32, kind="ExternalInput").ap()
    win_d = nc.dram_tensor("winT", [NX, HID], F32, kind="ExternalInput").ap()
    wout_d = nc.dram_tensor("woutC", [128, KC], F32, kind="ExternalInput").ap()
    bin_d = nc.dram_tensor("binC", [128, KC], F32, kind="ExternalInput").ap()
    bg_d = nc.dram_tensor("bgC", [128, MC], F32, kind="ExternalInput").ap()
    bout_d = nc.dram_tensor("bout", [1, 1], F32, kind="ExternalInput").ap()
    y_d = nc.dram_tensor("y", [nt, G], F32, kind="ExternalOutput").ap()

    WDT = mybir.dt.bfloat16 if bf16 else F32R
    from contextlib import ExitStack

    with tile.TileContext(nc) as tc, ExitStack() as ctx:
        cpool = ctx.enter_context(tc.tile_pool(name="const", bufs=1))
        stag_pool = ctx.enter_context(tc.tile_pool(name="stag", bufs=2))
        xt_pool = ctx.enter_context(tc.tile_pool(name="xt", bufs=3))
        xtr_pool = ctx.enter_context(tc.tile_pool(name="xtr", bufs=3))
        x0_pool = ctx.enter_context(tc.tile_pool(name="x0", bufs=12))
        act_pool = ctx.enter_context(tc.tile_pool(name="acts", bufs=14))
        tmp_pool = ctx.enter_context(tc.tile_pool(name="tmp", bufs=6))
        state_pool = ctx.enter_context(tc.tile_pool(name="state", bufs=1))
        y_pool = ctx.enter_context(tc.tile_pool(name="ysb", bufs=2))
        if shared_psum:
            pspool = ctx.enter_context(
                tc.tile_pool(name="pspool", bufs=8, space="PSUM"))
            gpsum = xpsum = ypsum = pspool
            g_tag = x_tag = y_tag = "ps"
        else:
            gpsum = ctx.enter_context(tc.tile_pool(
                name="gpsum", bufs=6 if psum6 else 5, space="PSUM"))
            xpsum = ctx.enter_context(tc.tile_pool(
                name="xpsum", bufs=1 if psum6 else 2, space="PSUM"))
            ypsum = ctx.enter_context(tc.tile_pool(
                name="ypsum", bufs=1, space="PSUM"))
            g_tag, x_tag, y_tag = "gps", "xps", "yps"
        if True:
            # ---- load + round weights ----
            w_ih_r = cpool.tile([128, KC * NGATE], WDT, name="w_ih_r", tag="w_ih_r")
            w_hh_r = cpool.tile([128, KC * NGATE], WDT, name="w_hh_r", tag="w_hh_r")
            for k in range(KC):
                st = stag_pool.tile([128, NGATE], F32, name="st", tag="st")
                nc.gpsimd.dma_start(st[:], wih_d[ts(k, 128), :])
                nc.vector.tensor_copy(w_ih_r[:, ts(k, NGATE)], st[:])
                st2 = stag_pool.tile([128, NGATE], F32, name="st", tag="st")
                nc.gpsimd.dma_start(st2[:], whh_d[ts(k, 128), :])
                nc.vector.tensor_copy(w_hh_r[:, ts(k, NGATE)], st2[:])
            stw = stag_pool.tile([NX, HID], F32, name="stw", tag="stw")
            nc.gpsimd.dma_start(stw[:], win_d[:, :])
            w_in_r = cpool.tile([NX, HID], WDT, name="w_in_r", tag="w_in_r")
            nc.vector.tensor_copy(w_in_r[:], stw[:])
            sto = stag_pool.tile([128, KC], F32, name="sto", tag="sto")
            nc.gpsimd.dma_start(sto[:], wout_d[:, :])
            w_out_r = cpool.tile([128, KC], WDT, name="w_out_r", tag="w_out_r")
            nc.vector.tensor_copy(w_out_r[:], sto[:])
            b_in_sb = cpool.tile([128, KC], F32, name="b_in_sb", tag="b_in_sb")
            nc.gpsimd.dma_start(b_in_sb[:], bin_d[:, :])
            b_g_sb = cpool.tile([128, MC], F32, name="b_g_sb", tag="b_g_sb")
            nc.gpsimd.dma_start(b_g_sb[:], bg_d[:, :])
            b_out_sb = cpool.tile([1, 1], F32, name="b_out_sb", tag="b_out_sb")
            nc.gpsimd.dma_start(b_out_sb[:], bout_d[:, :])

            # ---- persistent state (h double-buffered: gate matmuls of step t
            # read h from step t-1 while the elementwise tail writes step t's) ----
            h_a = [state_pool.tile([128, G], WDT, name=f"ha{j}", tag=f"ha{j}") for j in range(KC)]
            h_b = [state_pool.tile([128, G], WDT, name=f"hb{j}", tag=f"hb{j}") for j in range(KC)]
            c_t = [state_pool.tile([128, G], F32, name=f"c{j}", tag=f"c{j}") for j in range(KC)]

            gate_funcs = [AF.Sigmoid, AF.Sigmoid, AF.Tanh, AF.Sigmoid]  # i, f, g, o

            def emit_y(t_out, h_src):
                # y_{t_out} = h_{t_out} @ W_out + b_out, computed from h_src
                yps = ypsum.tile([1, G], F32, name="yps", tag=y_tag)
                for k in range(KC):
                    nc.tensor.matmul(
                        yps[:],
                        w_out_r[:, k : k + 1],
                        h_src[k][:],
                        start=(k == 0),
                        stop=(k == KC - 1),
                    )
                y_sb = y_pool.tile([1, G], F32, name="y_sb", tag="y_sb")
                nc.scalar.activation(y_sb[:], yps[:], AF.Identity, bias=b_out_sb[:, 0:1])
                nc.gpsimd.dma_start(y_d[ds(t_out, 1)] if isinstance(t_out, int) else y_d[ds(t_out, 1)], y_sb[:])

            def step(t, first, h_prev, h_cur):
                if not first:
                    # y_{t-1} from h_prev: identical values, but schedulable
                    # early instead of extending step t-1's serial tail.
                    emit_y(t - 1, h_prev)
                xt_sb = xt_pool.tile([NX, G], XDT, name="xt_sb", tag="xt_sb")
                xt_src = xt_d[ds(0, NX), :] if bench_mode else xt_d[ts(t, NX), :]
                nc.gpsimd.dma_start(xt_sb[:], xt_src)
                if bf16:
                    xt_r = xt_sb
                else:
                    xt_r = xtr_pool.tile([NX, G], WDT, name="xt_r", tag="xt_r")
                    nc.vector.tensor_copy(xt_r[:], xt_sb[:])

                x0_r = []
                for m in range(KC):
                    xps = xpsum.tile([128, G], F32, name="xps", tag=x_tag)
                    nc.tensor.matmul(
                        xps[:], w_in_r[:, ts(m, 128)], xt_r[:], start=True, stop=True
                    )
                    xr = x0_pool.tile([128, G], WDT, name="x0r", tag="x0r")
                    nc.scalar.activation(
                        xr[:], xps[:], AF.Relu, bias=b_in_sb[:, m : m + 1]
                    )
                    x0_r.append(xr)

                for j in range(KC):
                    acts = []
                    for gi in range(4):
                        m = gi * KC + j
                        gps = gpsum.tile([128, G], F32, name="gps", tag=g_tag)
                        mms = [
                            (w_ih_r[:, ds(k * NGATE + m * 128, 128)], x0_r[k])
                            for k in range(KC)
                        ]
                        if not first:
                            mms += [
                                (w_hh_r[:, ds(k * NGATE + m * 128, 128)], h_prev[k])
                                for k in range(KC)
                            ]
                        for idx, (lhsT, rhs) in enumerate(mms):
                            nc.tensor.matmul(
                                gps[:],
                                lhsT,
                                rhs[:],
                                start=(idx == 0),
                                stop=(idx == len(mms) - 1),
                            )
                        a = act_pool.tile([128, G], F32, name="act", tag="act")
                        nc.scalar.activation(
                            a[:], gps[:], gate_funcs[gi], bias=b_g_sb[:, m : m + 1]
                        )
                        acts.append(a)
                    a_i, a_f, a_g, a_o = acts
                    if first:
                        nc.vector.tensor_mul(c_t[j][:], a_i[:], a_g[:])
                    else:
                        t1 = tmp_pool.tile([128, G], F32, name="t1", tag="t1")
                        nc.vector.tensor_mul(t1[:], a_f[:], c_t[j][:])
                        t2 = tmp_pool.tile([128, G], F32, name="t2", tag="t2")
                        nc.vector.tensor_mul(t2[:], a_i[:], a_g[:])
                        nc.vector.tensor_add(c_t[j][:], t1[:], t2[:])
                    tanc = tmp_pool.tile([128, G], F32, name="tanc", tag="tanc")
                    nc.scalar.activation(tanc[:], c_t[j][:], AF.Tanh)
                    nc.vector.tensor_mul(h_cur[j][:], a_o[:], tanc[:])


            def one_pass():
                step(0, first=True, h_prev=None, h_cur=h_a)
                # steps 1.. alternate: odd t reads h_a writes h_b, even t
                # reads h_b writes h_a  (t=0 wrote h_a)
                if use_loop:
                    assert (nt - 1) % unroll == 0 and unroll % 2 == 0
                    with tc.For_i(1, nt, unroll, staggered_reset=staggered) as iv:
                        for u in range(unroll):
                            hp, hc = (h_a, h_b) if u % 2 == 0 else (h_b, h_a)
                            step(iv + u, first=False, h_prev=hp, h_cur=hc)
                else:
                    for t in range(1, nt):
                        hp, hc = (h_a, h_b) if t % 2 == 1 else (h_b, h_a)
                        step(t, first=False, h_prev=hp, h_cur=hc)
                # last step's y ((nt-1) even -> h in h_a)
                emit_y(nt - 1, h_a if (nt - 1) % 2 == 0 else h_b)

            if reps == 1:
                one_pass()
            else:
                with tc.For_i(0, reps, 1):
                    one_pass()

    nc.compile()
    return nc


def _prep_inputs(nt, inputs, W_in, b_in, w_ih, w_hh, b_ih, b_hh, W_out, b_out):
    f = np.float32
    inputs = np.ascontiguousarray(np.asarray(inputs, f))
    wihT = np.ascontiguousarray(np.asarray(w_ih, f).T)  # [HID, 4H]
    whhT = np.ascontiguousarray(np.asarray(w_hh, f).T)
    winT = np.ascontiguousarray(np.asarray(W_in, f).T)  # [NX, HID]
    woutC = np.ascontiguousarray(np.asarray(W_out, f).reshape(NY, KC, 128)[0].T)
    binC = np.ascontiguousarray(np.asarray(b_in, f).reshape(KC, 128).T)
    bgC = np.ascontiguousarray(
        (np.asarray(b_ih, f) + np.asarray(b_hh, f)).reshape(MC, 128).T
    )
    bout = np.asarray(b_out, f).reshape(1, 1)
    shared = dict(
        wihT=wihT, whhT=whhT, winT=winT, woutC=woutC, binC=binC, bgC=bgC, bout=bout
    )
    import ml_dtypes
    in_maps = []
    for c in range(N_CORES):
        xc = inputs[:nt, c * G : (c + 1) * G, :]  # [nt, G, NX]
        xt = np.ascontiguousarray(xc.transpose(0, 2, 1)).reshape(nt * NX, G)
        xt = xt.astype(ml_dtypes.bfloat16)
        in_maps.append({"xt": xt, **shared})
    return in_maps


def run(inputs_dict, trace=False, nt=NT, unroll=U, use_loop=True, staggered=False, psum6=False, **spmd_kwargs):
    nc = build_program(nt, unroll, use_loop, staggered, psum6)
    in_maps = _prep_inputs(nt, **inputs_dict)
    res = run_bass_kernel_spmd(
        nc, in_maps, core_ids=list(range(N_CORES)), trace=trace, **spmd_kwargs
    )
    out = np.empty((nt, NGRID, NY), np.float32)
    for c in range(N_CORES):
        out[:, c * G : (c + 1) * G, 0] = res.results[c]["y"]
    return out, res


def kernel(**inputs):
    out, _ = run(inputs, trace=False)
    return out

